# revision 12
# baseline (speedup 1.0000x reference)
"""NCNPredictor Trainium2 kernel: bit-packed adjacency + 3-channel EQ extraction.

out[e] = xij(e) + sum_n [ yA(n)*(b0&~b1) + yB(n)*b1 + yC(n)*(b2&~b0) ] + b
with b0 = a01[i,n]*a01[j,n], b1 = a1[..], b2 = a012[..]. Using the identity
b0b1 = b0&b1 and b0b2 = b0&b2 (bits!), the reference's 5 bilinear channels
collapse to 3 per column: after w = gi & gj (per-side 3-bit codes p,q,r at
bits 0-2, two adjacency columns per int16 lane: even col bits 0-2, odd col
bits 8-10),
   cA = (w&3)==1,  cB = (w&2) in {0,2},  cC = (w&5)==4.
Each extraction is ONE tensor_scalar (op0=AND, op1=is_equal) running in 4x
DVE mode; each product mask*y is a 2x tensor_tensor; each row-sum runs on the
ACT engine via activation(Copy, accum_out) (5 channels) or on the Pool engine
via tensor_reduce (1 channel), so all three elementwise engines stay busy.
The 0.5 scale of cB and channel signs are folded into the host-precomputed y
vectors. y vectors (x @ Wxs blocks), xij dot products, and the bias are
precomputed on the host, mirroring the reference's host-side weight algebra.

Sharding: target edges split across the 8 cores (1024 each); each core scans
all N adjacency columns of its own edges, so no cross-core reduction.
"""

import sys
from contextlib import ExitStack

import numpy as np

sys.path.insert(0, "/opt/trn_rl_repo")

import concourse.bass as bass
import concourse.tile as tile
from concourse import bacc, mybir
from concourse.bass_utils import run_bass_kernel_spmd

N = 10000
D = 128
E = 8192
NCORES = 8
E_OWN = E // NCORES          # 1024 edges per core
P = 128
T = E_OWN // P               # 8 tiles per core
WL = N // 2                  # 5000 int16 lanes of packed adjacency
F32 = mybir.dt.float32
BF16 = mybir.dt.bfloat16
I16 = mybir.dt.int16
I32 = mybir.dt.int32
MUL = mybir.AluOpType.mult
ADD = mybir.AluOpType.add
AND = mybir.AluOpType.bitwise_and
EQ = mybir.AluOpType.is_equal

# Channel extraction programs. The walrus verifier forbids mixing bitwise and
# arith ops inside one two-op tensor_scalar, so even-parity channels use
# arith-only (mod/compare) fused pairs on the low bits and odd-parity channels
# split into a single-op AND followed by a single-op is_equal. Each entry:
# (y index 0=A,1=B,2=C, parity, y scale, [steps]), step = (op0, imm_scalar1
# or ('bitc', idx), op1 or None, imm_scalar2).
CHANNELS = [
    (0, 0, 1.0, [(AND, ("bitc", 0), None, None),
                 (EQ, 1.0, None, None)]),                     # A even: (w&3)==1
    (1, 0, 0.5, [(AND, ("bitc", 1), None, None)]),            # B even: {0,2}
    (2, 0, 1.0, [(AND, ("bitc", 2), None, None),
                 (EQ, 4.0, None, None)]),                     # C even: (w&5)==4
    (0, 1, 1.0, [(AND, ("bitc", 3), None, None),
                 (EQ, 256.0, None, None)]),                   # A odd
    (1, 1, 1.0 / 512.0, [(AND, ("bitc", 4), None, None)]),    # B odd: {0,512}
    (2, 1, 1.0, [(AND, ("bitc", 5), None, None),
                 (EQ, 1024.0, None, None)]),                  # C odd
]
BITC_AND = [0x0003, 0x0002, 0x0005, 0x0300, 0x0200, 0x0500]
NBC = len(BITC_AND)

POOL_MUL_CH = 3               # A-odd product (bf16 x bf16) runs on Pool

POOL_SUM_CH = 5               # channel whose row-sum runs on the Pool engine

_CACHE = {}


def _build_nc(reps=1):
    nc = bacc.Bacc(num_swdge_queues=4)

    tableA = nc.declare_dram_parameter("tableA", [N, WL], I16, False)
    ycat = nc.declare_dram_parameter("ycat", [P, 6 * WL], BF16, False)
    bitc = nc.declare_dram_parameter("bitc", [P, NBC], I16, False)
    iall = nc.declare_dram_parameter("iall", [P, T], I32, False)
    jall = nc.declare_dram_parameter("jall", [P, T], I32, False)
    outb = nc.declare_dram_parameter("outb", [P, T], F32, True)

    with tile.TileContext(nc) as tc, ExitStack() as ctx:
        const = ctx.enter_context(tc.tile_pool(name="const", bufs=1))
        yk = []
        for u in range(6):
            y_t = const.tile([P, WL], BF16, name=f"y{u}")
            nc.sync.dma_start(y_t[:], ycat[:, u * WL : (u + 1) * WL])
            yk.append(y_t)
        bitc_t = const.tile([P, NBC], I16, name="bitc_t")
        nc.sync.dma_start(bitc_t[:], bitc[:])
        iall_t = const.tile([P, T], I32, name="iall_t")
        nc.sync.dma_start(iall_t[:], iall[:])
        jall_t = const.tile([P, T], I32, name="jall_t")
        nc.sync.dma_start(jall_t[:], jall[:])

        ao_act = const.tile([P, WL], BF16, name="ao_act")
        ao_pool = const.tile([P, WL], BF16, name="ao_pool")

        gip = ctx.enter_context(tc.tile_pool(name="gip", bufs=2))
        gjp = ctx.enter_context(tc.tile_pool(name="gjp", bufs=2))
        wp = ctx.enter_context(tc.tile_pool(name="wp", bufs=2))
        scrd = ctx.enter_context(tc.tile_pool(name="scrd", bufs=2))
        accp = ctx.enter_context(tc.tile_pool(name="accp", bufs=2))
        outp = ctx.enter_context(tc.tile_pool(name="outp", bufs=1))

        outb_t = outp.tile([P, T], F32, name="outb_t")

        for t in range(T * reps):
            t = t % T
            ioff = bass.IndirectOffsetOnAxis(ap=iall_t[:, t : t + 1], axis=0)
            joff = bass.IndirectOffsetOnAxis(ap=jall_t[:, t : t + 1], axis=0)

            gi = gip.tile([P, WL], I16, name="gi")
            nc.gpsimd.indirect_dma_start(
                out=gi[:], out_offset=None, in_=tableA[:], in_offset=ioff)
            gj = gjp.tile([P, WL], I16, name="gj")
            nc.gpsimd.indirect_dma_start(
                out=gj[:], out_offset=None, in_=tableA[:], in_offset=joff)
            w = wp.tile([P, WL], I16, name="w")
            nc.vector.tensor_tensor(out=w[:], in0=gi[:], in1=gj[:], op=AND)

            acc = accp.tile([P, 6], F32, name="acc")
            for k, (yi, par, _scale, steps) in enumerate(CHANNELS):
                cur = w
                for si, (op0, s1, op1, s2) in enumerate(steps):
                    last = si == len(steps) - 1
                    odt = I16 if (op0 == AND and last and op1 is None) else (
                        I16 if not last else BF16)
                    m = scrd.tile([P, WL], odt, name="mi" if odt == I16 else "mf")
                    if isinstance(s1, tuple):
                        s1v = bitc_t[:, s1[1] : s1[1] + 1]
                    else:
                        s1v = s1
                    nc.vector.tensor_scalar(
                        out=m[:], in0=cur[:], scalar1=s1v,
                        scalar2=s2, op0=op0,
                        **({} if op1 is None else {"op1": op1}),
                    )
                    cur = m
                s = scrd.tile([P, WL], BF16, name="s")
                eng = nc.gpsimd if k == POOL_MUL_CH else nc.vector
                eng.tensor_tensor(
                    out=s[:], in0=cur[:], in1=yk[par * 3 + yi][:], op=MUL,
                )
                nc.scalar.activation(
                    out=ao_act[:], in_=s[:],
                    func=mybir.ActivationFunctionType.Copy,
                    bias=0.0, scale=1.0,
                    accum_out=acc[:, k : k + 1],
                )
            nc.vector.tensor_reduce(
                out=outb_t[:, t : t + 1], in_=acc[:], axis=mybir.AxisListType.X,
                op=ADD,
            )

        nc.sync.dma_start(outb[:], outb_t[:])

    return nc


def get_nc(reps=1):
    key = f"nc{reps}"
    if key not in _CACHE:
        nc = _build_nc(reps)
        nc.compile()
        _CACHE[key] = nc
    return _CACHE[key]


def make_in_maps(x, adj_0_1, adj_1, adj_0_1_2, tar_ei, Wxs, bxs):
    import ml_dtypes

    bf = ml_dtypes.bfloat16
    x32 = np.ascontiguousarray(x, dtype=np.float32)
    wxs = np.asarray(Wxs, dtype=np.float32)
    w0 = wxs[0:D, 0]
    Y = x32 @ np.concatenate(
        [wxs[D : 2 * D], wxs[2 * D : 3 * D], wxs[3 * D : 4 * D]], axis=1
    )  # [N, 3] f32

    p = (np.asarray(adj_0_1) != 0)
    q = (np.asarray(adj_1) != 0)
    r = (np.asarray(adj_0_1_2) != 0)
    bits = (
        p.astype(np.uint8)
        | (q.astype(np.uint8) << 1)
        | (r.astype(np.uint8) << 2)
    )
    tableA = (
        bits[:, 0::2].astype(np.uint16) | (bits[:, 1::2].astype(np.uint16) << 8)
    ).view(np.int16)  # [N, WL]

    # per-channel y scale folded on host (e.g. B-odd mask value 512)
    scale_of = {}
    for yi_, par_, sc_, _steps in CHANNELS:
        scale_of[(par_, yi_)] = sc_
    ycat = np.empty((P, 6 * WL), dtype=bf)
    for par in range(2):
        for yi in range(3):
            u = par * 3 + yi
            col = (Y[par::2, yi] * scale_of[(par, yi)]).astype(bf)
            ycat[:, u * WL : (u + 1) * WL] = col[None, :]

    bitc = np.broadcast_to(
        np.asarray(BITC_AND, dtype=np.uint16)[None, :].view(np.int16), (P, NBC)
    ).copy()

    ii = np.asarray(tar_ei[0], dtype=np.int32)
    jj = np.asarray(tar_ei[1], dtype=np.int32)

    # xij term on the host (tiny E x D gather-dot), added in combine_results
    xw = x32 * w0[None, :]
    xij = np.einsum(
        "ed,ed->e", xw[ii].astype(np.float32), x32[jj].astype(np.float32)
    ).astype(np.float64)

    # Per-core edge sort by source row: i-side gathers walk the table in
    # ascending row order (better HBM page locality). Un-permuted in
    # combine_results.
    in_maps = []
    perms = []
    for c in range(NCORES):
        esl = slice(c * E_OWN, (c + 1) * E_OWN)
        ic, jc = ii[esl], jj[esl]
        order = np.argsort(ic, kind="stable")
        perms.append(order)
        in_maps.append({
            "tableA": tableA,
            "ycat": ycat,
            "bitc": bitc,
            "iall": np.ascontiguousarray(ic[order].reshape(T, P).T),
            "jall": np.ascontiguousarray(jc[order].reshape(T, P).T),
        })
    _CACHE["xij"] = xij
    _CACHE["perms"] = perms
    return in_maps


def combine_results(results, b):
    parts = []
    for c in range(NCORES):
        r = np.asarray(results[c]["outb"], dtype=np.float64)  # [P, T]
        vals = r.T.reshape(E_OWN)  # in sorted-edge order
        unperm = np.empty(E_OWN, dtype=np.float64)
        unperm[_CACHE["perms"][c]] = vals
        parts.append(unperm)
    out = np.concatenate(parts) + _CACHE["xij"] + b
    return out.astype(np.float32).reshape(E, 1)


def kernel(x, adj_0_1, adj_1, adj_0_1_2, tar_ei, Wxs, bxs):
    nc = get_nc()
    in_maps = make_in_maps(x, adj_0_1, adj_1, adj_0_1_2, tar_ei, Wxs, bxs)
    res = run_bass_kernel_spmd(nc, in_maps, list(range(NCORES)))
    b = float(np.asarray(bxs, dtype=np.float32).reshape(-1)[0])
    return combine_results(res.results, b)
